# revision 10
# baseline (speedup 1.0000x reference)
"""Trainium2 Bass kernel for nn_AttentionModel (B=4,S=2048,H=8,E=64, dropout mask).

Sharding: the 32 (b,h) pairs over 8 cores (4 pairs/core). All device compute is
in the *transposed* orientation scoresT[t,s] so the PV matmul consumes probsT
directly with no big on-chip transposes:

  qTproj[f,s] = Wq_aug.T @ qT_aug      (K=65: 64 e-rows + host-appended ones row)
  scoresT[t,s] = kTproj[:,t].T @ qTproj[:,s]     (K=64, fp16)
  expT = exp(scoresT/8)  (ACT, PSUM->SBUF, fp16)
  den[s] = ones.T @ expT                (PE ones-matmul, fp32 accum)
  probsT = expT * maskT                 (DVE fp16 2x mode)
  outT[e,s] += vproj[t,:].T @ probsT    (PE, fp16)
  out[s,e] = transpose(outT) * (1/(0.9*den[s]))   (PE transpose + DVE scale)

den/PV run DEPTH iterations behind scores/exp (software pipeline) so the PE
FIFO never stalls waiting on ACT/DVE. Host side only does layout prep
(transpose / fp16 cast / shard / gather).
"""

import os
import sys

sys.path.insert(0, "/opt/trn_rl_repo")

import numpy as np

import concourse.bass as bass
import concourse.mybir as mybir
import concourse.tile as tile
from concourse import bacc, bass_utils
from concourse.bass import ds, ts
from concourse.masks import make_identity

B, S, H, E = 4, 2048, 8, 64
E1 = E + 1                 # augmented contraction (ones/bias row)
NCORES = 8
PAIRS = (B * H) // NCORES  # 4 (b,h) pairs per core
SC = 1024                  # s-chunk width
NSC = S // SC              # 2
NTT = S // 128             # 16 t-tiles
DEPTH = 2                  # den/pv pipeline delay (iterations)
F32 = mybir.dt.float32
FP16 = mybir.dt.float16
INV_KEEP = 1.0 / 0.9

_CACHED_NC = None


def _body(tc, qT_d, kT_d, vT_d, mT_d, wq_d, wk_d, wv_d, out_d):
    nc = tc.nc
    Exp = mybir.ActivationFunctionType.Exp
    with (
        tc.tile_pool(name="const", bufs=1) as const,
        tc.tile_pool(name="io", bufs=2) as io,
        tc.tile_pool(name="proj", bufs=2) as proj,
        tc.tile_pool(name="work", bufs=2 + DEPTH) as work,
        tc.tile_pool(name="fin", bufs=2) as fin,
        tc.tile_pool(name="psA", bufs=2, space=bass.MemorySpace.PSUM) as psA,
        tc.tile_pool(name="psB", bufs=1, space=bass.MemorySpace.PSUM) as psB,
        tc.tile_pool(name="psD", bufs=1, space=bass.MemorySpace.PSUM) as psD,
    ):
        # --- constants ---
        wq = const.tile([E1, E], FP16, tag="wq")
        wk = const.tile([E1, E], FP16, tag="wk")
        wv = const.tile([E1, E], FP16, tag="wv")
        nc.sync.dma_start(wq[:, :], wq_d[:, :])
        nc.sync.dma_start(wk[:, :], wk_d[:, :])
        nc.sync.dma_start(wv[:, :], wv_d[:, :])
        ident = const.tile([E, E], F32, tag="ident")
        make_identity(nc, ident[:, :])
        ones = const.tile([128, 1], FP16, tag="ones")
        nc.vector.memset(ones[:, :], 1.0)
        zbias = const.tile([128, 1], F32, tag="zbias")
        nc.vector.memset(zbias[:, :], 0.0)

        def load_pair(p):
            qt = io.tile([E1, S], FP16, tag="qt", name="qt")
            kt = io.tile([E1, S], FP16, tag="kt", name="kt")
            vt = io.tile([E1, S], FP16, tag="vt", name="vt")
            nc.sync.dma_start(qt[:, :], qT_d[p])
            nc.sync.dma_start(kt[:, :], kT_d[p])
            nc.sync.dma_start(vt[:, :], vT_d[p])
            return qt, kt, vt

        def proj_ops(p, qt, kt, vt):
            """Projection as a list of closures, one emitted per main-loop
            iteration of the previous pair (keeps the PE FIFO dense)."""
            qp = proj.tile([E, S], FP16, tag="qp", name="qp")
            kp = proj.tile([E, S], FP16, tag="kp", name="kp")
            vp = proj.tile([128, NTT * E], FP16, tag="vp", name="vp")
            ops = []
            for c in range(S // 512):
                def qop(c=c):
                    pq = psA.tile([E, 512], F32, tag="scores", name="pq")
                    nc.tensor.matmul(pq[:, :], wq[:, :], qt[:, ts(c, 512)],
                                     start=True, stop=True)
                    nc.vector.tensor_copy(qp[:, ts(c, 512)], pq[:, :])
                def kop(c=c):
                    pk = psA.tile([E, 512], F32, tag="scores", name="pk")
                    nc.tensor.matmul(pk[:, :], wk[:, :], kt[:, ts(c, 512)],
                                     start=True, stop=True)
                    nc.vector.tensor_copy(kp[:, ts(c, 512)], pk[:, :])
                ops += [qop, kop]
            for t in range(NTT):
                def vop(t=t):
                    pv_ = psA.tile([128, E], F32, tag="scores", name="pv_")
                    nc.tensor.matmul(pv_[:, :], vt[:, ts(t, 128)], wv[:, :],
                                     start=True, stop=True)
                    nc.vector.tensor_copy(vp[:, ts(t, E)], pv_[:, :])
                ops.append(vop)
            return (qp, kp, vp), ops

        # --- flat schedule across pairs ---
        steps = [(c, t) for c in range(NSC) for t in range(NTT)]
        N = len(steps)
        LOAD_AT, PROJ_AT = 2, 6  # hooks inside the previous pair's main loop
        CDEL = 1                 # finalize copies: free PSUM asap
        FDEL = 6                 # finalize transposes/output: off the hot FIFO

        cur = load_pair(0)
        (qp, kp, vp), ops0 = proj_ops(0, *cur)
        for op in ops0:
            op()

        for p in range(PAIRS):
            exs, prs, dens, pvps, fins = {}, {}, {}, {}, {}
            nxt_tiles = None
            nxt_ops = []
            nxt_proj = None

            def finalize_copy(c):
                den, pvp = dens[c], pvps[c]
                drow = fin.tile([1, SC], F32, tag="drow", name="drow")
                nc.vector.tensor_copy(drow[:, :], den[:, :])
                dcol = fin.tile([128, SC // 128], F32, tag="dcol", name="dcol")
                for i in range(SC // 128):
                    nc.sync.dma_start(dcol[:, i : i + 1], drow[0:1, ts(i, 128)])
                inv = fin.tile([128, SC // 128], F32, tag="inv", name="inv")
                nc.vector.reciprocal(inv[:, :], dcol[:, :])
                nc.vector.tensor_scalar_mul(inv[:, :], inv[:, :], INV_KEEP)
                pvs = fin.tile([E, SC], F32, tag="pvs", name="pvs")
                nc.vector.tensor_copy(pvs[:, :], pvp[:, :])
                fins[c] = (inv, pvs)

            def finalize_out(p, c):
                inv, pvs = fins.pop(c)
                for st in range(SC // 128):
                    tp = psA.tile([128, E], F32, tag="scores", name="tp")
                    nc.tensor.transpose(tp[:, :], pvs[:, ts(st, 128)],
                                        ident[:, :])
                    ot = fin.tile([128, E], F32, tag="ot", name="ot", bufs=4)
                    nc.vector.tensor_scalar_mul(ot[:, :], tp[:, :],
                                                inv[:, st : st + 1])
                    nc.sync.dma_start(out_d[p, ds(c * SC + st * 128, 128), :],
                                      ot[:, :])

            for idx in range(N + DEPTH + FDEL + 1):
                if idx == LOAD_AT and p + 1 < PAIRS:
                    nxt_tiles = load_pair(p + 1)
                if idx == PROJ_AT and p + 1 < PAIRS:
                    nxt_proj, nxt_ops = proj_ops(p + 1, *nxt_tiles)
                if nxt_ops:
                    nxt_ops.pop(0)()
                if idx < N:
                    c, t = steps[idx]
                    if t == 0:
                        dens[c] = psD.tile([1, SC], F32, tag="den", name="den")
                        pvps[c] = psB.tile([E, SC], F32, tag="pv", name="pvp")
                    sp = psA.tile([128, SC], F32, tag="scores", name="sp")
                    nc.tensor.matmul(sp[:, 0:512], kp[:, ts(t, 128)],
                                     qp[:, ds(c * SC, 512)],
                                     start=True, stop=True)
                    nc.tensor.matmul(sp[:, 512:1024], kp[:, ts(t, 128)],
                                     qp[:, ds(c * SC + 512, 512)],
                                     start=True, stop=True)
                    ex = work.tile([128, SC], FP16, tag="ex", name="ex")
                    nc.scalar.activation(ex[:, :], sp[:, :], Exp,
                                         bias=zbias[:, :], scale=0.125)
                    mk = work.tile([128, SC], FP16, tag="mk", name="mk")
                    nc.sync.dma_start(mk[:, :],
                                      mT_d[p, ts(t, 128), ds(c * SC, SC)])
                    pr = work.tile([128, SC], FP16, tag="pr", name="pr")
                    nc.vector.tensor_mul(pr[:, :], ex[:, :], mk[:, :])
                    exs[idx], prs[idx] = ex, pr
                if DEPTH <= idx < N + DEPTH:
                    c, t = steps[idx - DEPTH]
                    ex, pr = exs.pop(idx - DEPTH), prs.pop(idx - DEPTH)
                    den, pvp = dens[c], pvps[c]
                    st0, stN = (t == 0), (t == NTT - 1)
                    for h in range(2):
                        sl = ds(h * 512, 512)
                        nc.tensor.matmul(den[:, sl], ones[:, :], ex[:, sl],
                                         start=st0, stop=stN)
                        nc.tensor.matmul(pvp[:, sl], vp[:, ts(t, E)],
                                         pr[:, sl], start=st0, stop=stN)
                    if stN:
                        finalize_copy(c)
                j = idx - DEPTH - FDEL
                if 0 <= j < N and steps[j][1] == NTT - 1:
                    finalize_out(p, steps[j][0])

            if p + 1 < PAIRS:
                for op in nxt_ops:
                    op()
                qp, kp, vp = nxt_proj


def _build():
    global _CACHED_NC
    if _CACHED_NC is not None:
        return _CACHED_NC
    nc = bacc.Bacc("TRN2", target_bir_lowering=False, debug=False,
                   num_devices=NCORES)
    qT_d = nc.dram_tensor("qT", [PAIRS, E1, S], FP16, kind="ExternalInput").ap()
    kT_d = nc.dram_tensor("kT", [PAIRS, E1, S], FP16, kind="ExternalInput").ap()
    vT_d = nc.dram_tensor("vT", [PAIRS, E1, S], FP16, kind="ExternalInput").ap()
    mT_d = nc.dram_tensor("maskT", [PAIRS, S, S], FP16, kind="ExternalInput").ap()
    wq_d = nc.dram_tensor("Wq", [E1, E], FP16, kind="ExternalInput").ap()
    wk_d = nc.dram_tensor("Wk", [E1, E], FP16, kind="ExternalInput").ap()
    wv_d = nc.dram_tensor("Wv", [E1, E], FP16, kind="ExternalInput").ap()
    out_d = nc.dram_tensor("out", [PAIRS, S, E], F32, kind="ExternalOutput").ap()
    with tile.TileContext(nc) as tc:
        _body(tc, qT_d, kT_d, vT_d, mT_d, wq_d, wk_d, wv_d, out_d)
    nc.compile()
    _CACHED_NC = nc
    return nc


def _aug(xT):
    """[n, E, S] -> [n, E+1, S] fp16 with a ones row appended."""
    n = xT.shape[0]
    out = np.empty((n, E1, S), np.float16)
    out[:, :E, :] = xT
    out[:, E, :] = 1.0
    return out


def _in_maps(inputs):
    query = np.asarray(inputs["query"], np.float32)
    key = np.asarray(inputs["key"], np.float32)
    value = np.asarray(inputs["value"], np.float32)
    mask = np.asarray(inputs["drop_mask"])
    # [B,S,H,E] -> [B*H, E, S], fp16, + ones row
    qT = _aug(query.transpose(0, 2, 3, 1).reshape(B * H, E, S))
    kT = _aug(key.transpose(0, 2, 3, 1).reshape(B * H, E, S))
    vT = _aug(value.transpose(0, 2, 3, 1).reshape(B * H, E, S))
    # [B,H,S,S] -> transposed [B*H, t, s] as fp16 {0,1}
    mT = (np.ascontiguousarray(mask.transpose(0, 1, 3, 2))
          .astype(np.float16).reshape(B * H, S, S))

    def waug(W, b):
        out = np.empty((E1, E), np.float16)
        out[:E, :] = np.asarray(W, np.float32)
        out[E, :] = np.asarray(b, np.float32).reshape(E)
        return out

    Wq = waug(inputs["Wq"], inputs["bq"])
    Wk = waug(inputs["Wk"], inputs["bk"])
    Wv = waug(inputs["Wv"], inputs["bv"])
    maps = []
    for c in range(NCORES):
        sl = slice(c * PAIRS, (c + 1) * PAIRS)
        maps.append({
            "qT": np.ascontiguousarray(qT[sl]),
            "kT": np.ascontiguousarray(kT[sl]),
            "vT": np.ascontiguousarray(vT[sl]),
            "maskT": np.ascontiguousarray(mT[sl]),
            "Wq": Wq, "Wk": Wk, "Wv": Wv,
        })
    return maps


def _gather(results):
    outs = [results[c]["out"] for c in range(NCORES)]
    return (np.concatenate(outs, axis=0)
            .reshape(B, H, S, E).astype(np.float32, copy=False))


def kernel(**inputs):
    nc = _build()
    maps = _in_maps(inputs)
    res = bass_utils.run_bass_kernel_spmd(nc, maps, core_ids=list(range(NCORES)))
    return _gather(res.results)


if __name__ == "__main__":
    _build()
    print("build+compile OK")


# revision 13
# speedup vs baseline: 1.0273x; 1.0273x over previous
"""Trainium2 Bass kernel for nn_AttentionModel (B=4,S=2048,H=8,E=64, dropout mask).

Sharding: the 32 (b,h) pairs over 8 cores (4 pairs/core). All device compute is
in the *transposed* orientation scoresT[t,s] so the PV matmul consumes probsT
directly with no big on-chip transposes:

  qTproj[f,s] = Wq_aug.T @ qT_aug      (K=65: 64 e-rows + host-appended ones row)
  scoresT[t,s] = kTproj[:,t].T @ qTproj[:,s]     (K=64, fp16)
  expT = exp(scoresT/8)  (ACT, PSUM->SBUF, fp16)
  den[s] = ones.T @ expT                (PE ones-matmul, fp32 accum)
  probsT = expT * maskT                 (DVE fp16 2x mode)
  outT[e,s] += vproj[t,:].T @ probsT    (PE, fp16)
  out[s,e] = transpose(outT) * (1/(0.9*den[s]))   (PE transpose + DVE scale)

den/PV run DEPTH iterations behind scores/exp (software pipeline) so the PE
FIFO never stalls waiting on ACT/DVE. Host side only does layout prep
(transpose / fp16 cast / shard / gather).
"""

import os
import sys

sys.path.insert(0, "/opt/trn_rl_repo")

import numpy as np

import concourse.bass as bass
import concourse.mybir as mybir
import concourse.tile as tile
from concourse import bacc, bass_utils
from concourse.bass import ds, ts
from concourse.masks import make_identity

B, S, H, E = 4, 2048, 8, 64
E1 = E + 1                 # augmented contraction (ones/bias row)
NCORES = 8
PAIRS = (B * H) // NCORES  # 4 (b,h) pairs per core
SC = 1024                  # s-chunk width
NSC = S // SC              # 2
NTT = S // 128             # 16 t-tiles
DEPTH = 2                  # den/pv pipeline delay (iterations)
F32 = mybir.dt.float32
FP16 = mybir.dt.float16
INV_KEEP = 1.0 / 0.9

_CACHED_NC = None


def _body(tc, qT_d, kT_d, vT_d, mT_d, wq_d, wk_d, wv_d, out_d):
    nc = tc.nc
    Exp = mybir.ActivationFunctionType.Exp
    with (
        tc.tile_pool(name="const", bufs=1) as const,
        tc.tile_pool(name="io", bufs=2) as io,
        tc.tile_pool(name="proj", bufs=2) as proj,
        tc.tile_pool(name="work", bufs=2 + DEPTH) as work,
        tc.tile_pool(name="fin", bufs=2) as fin,
        tc.tile_pool(name="psA", bufs=2, space=bass.MemorySpace.PSUM) as psA,
        tc.tile_pool(name="psB", bufs=1, space=bass.MemorySpace.PSUM) as psB,
        tc.tile_pool(name="psD", bufs=1, space=bass.MemorySpace.PSUM) as psD,
    ):
        # --- constants ---
        wq = const.tile([E1, E], FP16, tag="wq")
        wk = const.tile([E1, E], FP16, tag="wk")
        wv = const.tile([E1, E], FP16, tag="wv")
        nc.sync.dma_start(wq[:, :], wq_d[:, :])
        nc.sync.dma_start(wk[:, :], wk_d[:, :])
        nc.sync.dma_start(wv[:, :], wv_d[:, :])
        ident = const.tile([E, E], F32, tag="ident")
        make_identity(nc, ident[:, :])
        ones = const.tile([128, 1], FP16, tag="ones")
        nc.vector.memset(ones[:, :], 1.0)
        zbias = const.tile([128, 1], F32, tag="zbias")
        nc.vector.memset(zbias[:, :], 0.0)

        # --- prologue: load + project ALL pairs up front so the main loops
        # are uniform PE-limited 6-matmul iterations (HAM stays warm only
        # when the PE issues back-to-back). PSUM rotates over all pool tags.
        pslots = [(psA, "scores"), (psA, "scores"), (psB, "pv"), (psD, "den")]
        projd = []
        for p in range(PAIRS):
            qt = io.tile([E1, S], FP16, tag="qt", name="qt")
            kt = io.tile([E1, S], FP16, tag="kt", name="kt")
            vt = io.tile([E1, S], FP16, tag="vt", name="vt")
            nc.sync.dma_start(qt[:, :], qT_d[p])
            nc.sync.dma_start(kt[:, :], kT_d[p])
            nc.sync.dma_start(vt[:, :], vT_d[p])
            qp = proj.tile([E, S], FP16, tag="qp", name="qp", bufs=PAIRS)
            kp = proj.tile([E, S], FP16, tag="kp", name="kp", bufs=PAIRS)
            vp = proj.tile([128, NTT * E], FP16, tag="vp", name="vp",
                           bufs=PAIRS)
            rot = 0
            for w, dst, src in ((wq, qp, qt), (wk, kp, kt)):
                for c in range(S // 1024):
                    pool, tag = pslots[rot % 4]
                    rot += 1
                    pp = pool.tile([E, 1024], F32, tag=tag, name="pp")
                    nc.tensor.matmul(pp[:, 0:512], w[:, :],
                                     src[:, ds(c * 1024, 512)],
                                     start=True, stop=True)
                    nc.tensor.matmul(pp[:, 512:1024], w[:, :],
                                     src[:, ds(c * 1024 + 512, 512)],
                                     start=True, stop=True)
                    nc.vector.tensor_copy(dst[:, ds(c * 1024, 1024)], pp[:, :])
            for t in range(NTT):
                pool, tag = pslots[rot % 4]
                rot += 1
                pv_ = pool.tile([128, E], F32, tag=tag, name="pv_")
                nc.tensor.matmul(pv_[:, :], vt[:, ts(t, 128)], wv[:, :],
                                 start=True, stop=True)
                nc.vector.tensor_copy(vp[:, ts(t, E)], pv_[:, :])
            projd.append((qp, kp, vp))

        # --- main loops ---
        steps = [(c, t) for c in range(NSC) for t in range(NTT)]
        N = len(steps)
        FDEL = 6                 # finalize transposes/output: off the hot FIFO

        for p in range(PAIRS):
            qp, kp, vp = projd[p]
            exs, prs, dens, pvps, fins = {}, {}, {}, {}, {}

            def finalize_copy(c):
                den, pvp = dens[c], pvps[c]
                drow = fin.tile([1, SC], F32, tag="drow", name="drow")
                nc.vector.tensor_copy(drow[:, :], den[:, :])
                dcol = fin.tile([128, SC // 128], F32, tag="dcol", name="dcol")
                for i in range(SC // 128):
                    nc.sync.dma_start(dcol[:, i : i + 1], drow[0:1, ts(i, 128)])
                inv = fin.tile([128, SC // 128], F32, tag="inv", name="inv")
                nc.vector.reciprocal(inv[:, :], dcol[:, :])
                nc.vector.tensor_scalar_mul(inv[:, :], inv[:, :], INV_KEEP)
                pvs = fin.tile([E, SC], F32, tag="pvs", name="pvs")
                nc.vector.tensor_copy(pvs[:, :], pvp[:, :])
                fins[c] = (inv, pvs)

            def finalize_out(p, c):
                inv, pvs = fins.pop(c)
                for st in range(SC // 128):
                    tp = psA.tile([128, E], F32, tag="scores", name="tp")
                    nc.tensor.transpose(tp[:, :], pvs[:, ts(st, 128)],
                                        ident[:, :])
                    ot = fin.tile([128, E], F32, tag="ot", name="ot", bufs=4)
                    nc.vector.tensor_scalar_mul(ot[:, :], tp[:, :],
                                                inv[:, st : st + 1])
                    nc.sync.dma_start(out_d[p, ds(c * SC + st * 128, 128), :],
                                      ot[:, :])

            for idx in range(N + DEPTH + FDEL + 1):
                if idx < N:
                    c, t = steps[idx]
                    if t == 0:
                        dens[c] = psD.tile([1, SC], F32, tag="den", name="den")
                        pvps[c] = psB.tile([E, SC], F32, tag="pv", name="pvp")
                    sp = psA.tile([128, SC], F32, tag="scores", name="sp")
                    nc.tensor.matmul(sp[:, 0:512], kp[:, ts(t, 128)],
                                     qp[:, ds(c * SC, 512)],
                                     start=True, stop=True)
                    nc.tensor.matmul(sp[:, 512:1024], kp[:, ts(t, 128)],
                                     qp[:, ds(c * SC + 512, 512)],
                                     start=True, stop=True)
                    ex = work.tile([128, SC], FP16, tag="ex", name="ex")
                    nc.scalar.activation(ex[:, :], sp[:, :], Exp,
                                         bias=zbias[:, :], scale=0.125)
                    mk = work.tile([128, SC], FP16, tag="mk", name="mk")
                    nc.sync.dma_start(mk[:, :],
                                      mT_d[p, ts(t, 128), ds(c * SC, SC)])
                    pr = work.tile([128, SC], FP16, tag="pr", name="pr")
                    nc.vector.tensor_mul(pr[:, :], ex[:, :], mk[:, :])
                    exs[idx], prs[idx] = ex, pr
                if DEPTH <= idx < N + DEPTH:
                    c, t = steps[idx - DEPTH]
                    ex, pr = exs.pop(idx - DEPTH), prs.pop(idx - DEPTH)
                    den, pvp = dens[c], pvps[c]
                    st0, stN = (t == 0), (t == NTT - 1)
                    for h in range(2):
                        sl = ds(h * 512, 512)
                        nc.tensor.matmul(den[:, sl], ones[:, :], ex[:, sl],
                                         start=st0, stop=stN)
                        nc.tensor.matmul(pvp[:, sl], vp[:, ts(t, E)],
                                         pr[:, sl], start=st0, stop=stN)
                    if stN:
                        finalize_copy(c)
                j = idx - DEPTH - FDEL
                if 0 <= j < N and steps[j][1] == NTT - 1:
                    finalize_out(p, steps[j][0])


def _build():
    global _CACHED_NC
    if _CACHED_NC is not None:
        return _CACHED_NC
    nc = bacc.Bacc("TRN2", target_bir_lowering=False, debug=False,
                   num_devices=NCORES)
    qT_d = nc.dram_tensor("qT", [PAIRS, E1, S], FP16, kind="ExternalInput").ap()
    kT_d = nc.dram_tensor("kT", [PAIRS, E1, S], FP16, kind="ExternalInput").ap()
    vT_d = nc.dram_tensor("vT", [PAIRS, E1, S], FP16, kind="ExternalInput").ap()
    mT_d = nc.dram_tensor("maskT", [PAIRS, S, S], FP16, kind="ExternalInput").ap()
    wq_d = nc.dram_tensor("Wq", [E1, E], FP16, kind="ExternalInput").ap()
    wk_d = nc.dram_tensor("Wk", [E1, E], FP16, kind="ExternalInput").ap()
    wv_d = nc.dram_tensor("Wv", [E1, E], FP16, kind="ExternalInput").ap()
    out_d = nc.dram_tensor("out", [PAIRS, S, E], F32, kind="ExternalOutput").ap()
    with tile.TileContext(nc) as tc:
        _body(tc, qT_d, kT_d, vT_d, mT_d, wq_d, wk_d, wv_d, out_d)
    nc.compile()
    _CACHED_NC = nc
    return nc


def _aug(xT):
    """[n, E, S] -> [n, E+1, S] fp16 with a ones row appended."""
    n = xT.shape[0]
    out = np.empty((n, E1, S), np.float16)
    out[:, :E, :] = xT
    out[:, E, :] = 1.0
    return out


def _in_maps(inputs):
    query = np.asarray(inputs["query"], np.float32)
    key = np.asarray(inputs["key"], np.float32)
    value = np.asarray(inputs["value"], np.float32)
    mask = np.asarray(inputs["drop_mask"])
    # [B,S,H,E] -> [B*H, E, S], fp16, + ones row
    qT = _aug(query.transpose(0, 2, 3, 1).reshape(B * H, E, S))
    kT = _aug(key.transpose(0, 2, 3, 1).reshape(B * H, E, S))
    vT = _aug(value.transpose(0, 2, 3, 1).reshape(B * H, E, S))
    # [B,H,S,S] -> transposed [B*H, t, s] as fp16 {0,1}
    mT = (np.ascontiguousarray(mask.transpose(0, 1, 3, 2))
          .astype(np.float16).reshape(B * H, S, S))

    def waug(W, b):
        out = np.empty((E1, E), np.float16)
        out[:E, :] = np.asarray(W, np.float32)
        out[E, :] = np.asarray(b, np.float32).reshape(E)
        return out

    Wq = waug(inputs["Wq"], inputs["bq"])
    Wk = waug(inputs["Wk"], inputs["bk"])
    Wv = waug(inputs["Wv"], inputs["bv"])
    maps = []
    for c in range(NCORES):
        sl = slice(c * PAIRS, (c + 1) * PAIRS)
        maps.append({
            "qT": np.ascontiguousarray(qT[sl]),
            "kT": np.ascontiguousarray(kT[sl]),
            "vT": np.ascontiguousarray(vT[sl]),
            "maskT": np.ascontiguousarray(mT[sl]),
            "Wq": Wq, "Wk": Wk, "Wv": Wv,
        })
    return maps


def _gather(results):
    outs = [results[c]["out"] for c in range(NCORES)]
    return (np.concatenate(outs, axis=0)
            .reshape(B, H, S, E).astype(np.float32, copy=False))


def kernel(**inputs):
    nc = _build()
    maps = _in_maps(inputs)
    res = bass_utils.run_bass_kernel_spmd(nc, maps, core_ids=list(range(NCORES)))
    return _gather(res.results)


if __name__ == "__main__":
    _build()
    print("build+compile OK")


# revision 18
# speedup vs baseline: 1.1016x; 1.0724x over previous
"""Trainium2 Bass kernel for nn_AttentionModel (B=4,S=2048,H=8,E=64, dropout mask).

Sharding: the 32 (b,h) pairs over 8 cores (4 pairs/core). All device compute is
in the *transposed* orientation scoresT[t,s] so the PV matmul consumes probsT
directly with no big on-chip transposes:

  qTproj[f,s] = Wq_aug.T @ qT_aug      (K=65: 64 e-rows + host-appended ones row)
  scoresT[t,s] = kTproj[:,t].T @ qTproj[:,s]     (K=64, fp16)
  expT = exp(scoresT/8)  (ACT, PSUM->SBUF, fp16)
  den[s] = ones.T @ expT                (PE ones-matmul, fp32 accum)
  probsT = expT * maskT                 (DVE fp16 2x mode)
  outT[e,s] += vproj[t,:].T @ probsT    (PE, fp16)
  out[s,e] = transpose(outT) * (1/(0.9*den[s]))   (PE transpose + DVE scale)

den/PV run DEPTH iterations behind scores/exp (software pipeline) so the PE
FIFO never stalls waiting on ACT/DVE. Host side only does layout prep
(transpose / fp16 cast / shard / gather).
"""

import os
import sys

sys.path.insert(0, "/opt/trn_rl_repo")

import numpy as np

import concourse.bass as bass
import concourse.mybir as mybir
import concourse.tile as tile
from concourse import bacc, bass_utils
from concourse.bass import ds, ts
from concourse.masks import make_identity

B, S, H, E = 4, 2048, 8, 64
E1 = E + 1                 # augmented contraction (ones/bias row)
NCORES = 8
PAIRS = (B * H) // NCORES  # 4 (b,h) pairs per core
SC = 1024                  # s-chunk width
NSC = S // SC              # 2
NTT = S // 128             # 16 t-tiles
DEPTH = 4                  # den/pv pipeline delay (iterations)
F32 = mybir.dt.float32
FP16 = mybir.dt.float16
INV_KEEP = 1.0 / 0.9

_CACHED_NC = None


def _body(tc, qT_d, kT_d, vT_d, mT_d, wq_d, wk_d, wv_d, out_d):
    nc = tc.nc
    Exp = mybir.ActivationFunctionType.Exp
    with (
        tc.tile_pool(name="const", bufs=1) as const,
        tc.tile_pool(name="io", bufs=2) as io,
        tc.tile_pool(name="proj", bufs=2) as proj,
        tc.tile_pool(name="work", bufs=2 + DEPTH) as work,
        tc.tile_pool(name="fin", bufs=2) as fin,
        tc.tile_pool(name="psA", bufs=3, space=bass.MemorySpace.PSUM) as psA,
        tc.tile_pool(name="psB", bufs=1, space=bass.MemorySpace.PSUM) as psB,
    ):
        # --- constants ---
        wq = const.tile([E1, E], FP16, tag="wq")
        wk = const.tile([E1, E], FP16, tag="wk")
        wv = const.tile([E1, E], FP16, tag="wv")
        nc.sync.dma_start(wq[:, :], wq_d[:, :])
        nc.sync.dma_start(wk[:, :], wk_d[:, :])
        nc.sync.dma_start(wv[:, :], wv_d[:, :])
        ident = const.tile([E, E], F32, tag="ident")
        make_identity(nc, ident[:, :])
        ones = const.tile([128, 1], FP16, tag="ones")
        nc.vector.memset(ones[:, :], 1.0)
        zbias = const.tile([128, 1], F32, tag="zbias")
        nc.vector.memset(zbias[:, :], 0.0)

        # --- prologue: load + project ALL pairs up front so the main loops
        # are uniform PE-limited 6-matmul iterations (HAM stays warm only
        # when the PE issues back-to-back). PSUM rotates over all pool tags.
        pslots = [(psA, "scores"), (psA, "scores"), (psA, "scores"), (psB, "pv")]
        projd = []
        for p in range(PAIRS):
            qt = io.tile([E1, S], FP16, tag="qt", name="qt")
            kt = io.tile([E1, S], FP16, tag="kt", name="kt")
            vt = io.tile([E1, S], FP16, tag="vt", name="vt")
            nc.sync.dma_start(qt[:, :], qT_d[p])
            nc.sync.dma_start(kt[:, :], kT_d[p])
            nc.sync.dma_start(vt[:, :], vT_d[p])
            qp = proj.tile([E, S], FP16, tag="qp", name="qp", bufs=PAIRS)
            kp = proj.tile([E, S], FP16, tag="kp", name="kp", bufs=PAIRS)
            vp = proj.tile([128, NTT * E], FP16, tag="vp", name="vp",
                           bufs=PAIRS)
            rot = 0
            for w, dst, src in ((wq, qp, qt), (wk, kp, kt)):
                for c in range(S // 1024):
                    pool, tag = pslots[rot % 4]
                    rot += 1
                    pp = pool.tile([E, 1024], F32, tag=tag, name="pp")
                    nc.tensor.matmul(pp[:, 0:512], w[:, :],
                                     src[:, ds(c * 1024, 512)],
                                     start=True, stop=True)
                    nc.tensor.matmul(pp[:, 512:1024], w[:, :],
                                     src[:, ds(c * 1024 + 512, 512)],
                                     start=True, stop=True)
                    nc.vector.tensor_copy(dst[:, ds(c * 1024, 1024)], pp[:, :])
            for t in range(NTT):
                pool, tag = pslots[rot % 4]
                rot += 1
                pv_ = pool.tile([128, E], F32, tag=tag, name="pv_")
                nc.tensor.matmul(pv_[:, :], vt[:, ts(t, 128)], wv[:, :],
                                 start=True, stop=True)
                nc.vector.tensor_copy(vp[:, ts(t, E)], pv_[:, :])
            projd.append((qp, kp, vp))

        # --- main loops ---
        steps = [(c, t) for c in range(NSC) for t in range(NTT)]
        N = len(steps)
        FDEL = 6                 # finalize transposes/output: off the hot FIFO

        for p in range(PAIRS):
            qp, kp, vp = projd[p]
            exs, prs, dens, pvps, fins = {}, {}, {}, {}, {}

            def finalize_copy(c):
                pvd = pvps[c]
                drow = fin.tile([1, SC], F32, tag="drow", name="drow")
                nc.vector.tensor_copy(drow[:, :], pvd[E : E + 1, :])
                dcol = fin.tile([128, SC // 128], F32, tag="dcol", name="dcol")
                for i in range(SC // 128):
                    nc.sync.dma_start(dcol[:, i : i + 1], drow[0:1, ts(i, 128)])
                inv = fin.tile([128, SC // 128], F32, tag="inv", name="inv")
                nc.vector.reciprocal(inv[:, :], dcol[:, :])
                nc.vector.tensor_scalar_mul(inv[:, :], inv[:, :], INV_KEEP)
                pvs = fin.tile([E, SC], F32, tag="pvs", name="pvs")
                nc.vector.tensor_copy(pvs[:, :], pvd[0:E, :])
                fins[c] = (inv, pvs)

            def finalize_out(p, c):
                inv, pvs = fins.pop(c)
                for st in range(SC // 128):
                    tp = psA.tile([128, E], F32, tag="scores", name="tp")
                    nc.tensor.transpose(tp[:, :], pvs[:, ts(st, 128)],
                                        ident[:, :])
                    ot = fin.tile([128, E], F32, tag="ot", name="ot", bufs=4)
                    nc.vector.tensor_scalar_mul(ot[:, :], tp[:, :],
                                                inv[:, st : st + 1])
                    nc.sync.dma_start(out_d[p, ds(c * SC + st * 128, 128), :],
                                      ot[:, :])

            for idx in range(N + DEPTH + FDEL + 1):
                # den/pv of iteration idx-DEPTH first: adds slack between
                # exp(idx-psA_bufs) completing and scores(idx) needing a slot
                if DEPTH <= idx < N + DEPTH:
                    c, t = steps[idx - DEPTH]
                    ex, pr = exs.pop(idx - DEPTH), prs.pop(idx - DEPTH)
                    pvd = pvps[c]
                    st0, stN = (t == 0), (t == NTT - 1)
                    for h in range(2):
                        sl = ds(h * 512, 512)
                        nc.tensor.matmul(pvd[0:E, sl], vp[:, ts(t, E)],
                                         pr[:, sl], start=st0, stop=stN,
                                         tile_position=(0, 0))
                        nc.tensor.matmul(pvd[E : E + 1, sl], ones[:, :],
                                         ex[:, sl], start=st0, stop=stN,
                                         tile_position=(0, 64))
                    if stN:
                        finalize_copy(c)
                if idx < N:
                    c, t = steps[idx]
                    if t == 0:
                        # partitions 0..63: PV accum; partition 64: den accum
                        pvps[c] = psB.tile([E + 1, SC], F32, tag="pv",
                                           name="pvd")
                    sp = psA.tile([128, SC], F32, tag="scores", name="sp")
                    nc.tensor.matmul(sp[:, 0:512], kp[:, ts(t, 128)],
                                     qp[:, ds(c * SC, 512)],
                                     start=True, stop=True)
                    nc.tensor.matmul(sp[:, 512:1024], kp[:, ts(t, 128)],
                                     qp[:, ds(c * SC + 512, 512)],
                                     start=True, stop=True)
                    ex = work.tile([128, SC], FP16, tag="ex", name="ex")
                    nc.scalar.activation(ex[:, :], sp[:, :], Exp,
                                         bias=zbias[:, :], scale=0.125)
                    mk = work.tile([128, SC], FP16, tag="mk", name="mk")
                    nc.sync.dma_start(mk[:, :],
                                      mT_d[p, ts(t, 128), ds(c * SC, SC)])
                    pr = work.tile([128, SC], FP16, tag="pr", name="pr")
                    nc.vector.tensor_mul(pr[:, :], ex[:, :], mk[:, :])
                    exs[idx], prs[idx] = ex, pr
                j = idx - DEPTH - FDEL
                if 0 <= j < N and steps[j][1] == NTT - 1:
                    finalize_out(p, steps[j][0])


def _build():
    global _CACHED_NC
    if _CACHED_NC is not None:
        return _CACHED_NC
    nc = bacc.Bacc("TRN2", target_bir_lowering=False, debug=False,
                   num_devices=NCORES)
    qT_d = nc.dram_tensor("qT", [PAIRS, E1, S], FP16, kind="ExternalInput").ap()
    kT_d = nc.dram_tensor("kT", [PAIRS, E1, S], FP16, kind="ExternalInput").ap()
    vT_d = nc.dram_tensor("vT", [PAIRS, E1, S], FP16, kind="ExternalInput").ap()
    mT_d = nc.dram_tensor("maskT", [PAIRS, S, S], FP16, kind="ExternalInput").ap()
    wq_d = nc.dram_tensor("Wq", [E1, E], FP16, kind="ExternalInput").ap()
    wk_d = nc.dram_tensor("Wk", [E1, E], FP16, kind="ExternalInput").ap()
    wv_d = nc.dram_tensor("Wv", [E1, E], FP16, kind="ExternalInput").ap()
    out_d = nc.dram_tensor("out", [PAIRS, S, E], F32, kind="ExternalOutput").ap()
    with tile.TileContext(nc) as tc:
        _body(tc, qT_d, kT_d, vT_d, mT_d, wq_d, wk_d, wv_d, out_d)
    nc.compile()
    _CACHED_NC = nc
    return nc


def _aug(xT):
    """[n, E, S] -> [n, E+1, S] fp16 with a ones row appended."""
    n = xT.shape[0]
    out = np.empty((n, E1, S), np.float16)
    out[:, :E, :] = xT
    out[:, E, :] = 1.0
    return out


def _in_maps(inputs):
    query = np.asarray(inputs["query"], np.float32)
    key = np.asarray(inputs["key"], np.float32)
    value = np.asarray(inputs["value"], np.float32)
    mask = np.asarray(inputs["drop_mask"])
    # [B,S,H,E] -> [B*H, E, S], fp16, + ones row
    qT = _aug(query.transpose(0, 2, 3, 1).reshape(B * H, E, S))
    kT = _aug(key.transpose(0, 2, 3, 1).reshape(B * H, E, S))
    vT = _aug(value.transpose(0, 2, 3, 1).reshape(B * H, E, S))
    # [B,H,S,S] -> transposed [B*H, t, s] as fp16 {0,1}
    mT = (np.ascontiguousarray(mask.transpose(0, 1, 3, 2))
          .astype(np.float16).reshape(B * H, S, S))

    def waug(W, b):
        out = np.empty((E1, E), np.float16)
        out[:E, :] = np.asarray(W, np.float32)
        out[E, :] = np.asarray(b, np.float32).reshape(E)
        return out

    Wq = waug(inputs["Wq"], inputs["bq"])
    Wk = waug(inputs["Wk"], inputs["bk"])
    Wv = waug(inputs["Wv"], inputs["bv"])
    maps = []
    for c in range(NCORES):
        sl = slice(c * PAIRS, (c + 1) * PAIRS)
        maps.append({
            "qT": np.ascontiguousarray(qT[sl]),
            "kT": np.ascontiguousarray(kT[sl]),
            "vT": np.ascontiguousarray(vT[sl]),
            "maskT": np.ascontiguousarray(mT[sl]),
            "Wq": Wq, "Wk": Wk, "Wv": Wv,
        })
    return maps


def _gather(results):
    outs = [results[c]["out"] for c in range(NCORES)]
    return (np.concatenate(outs, axis=0)
            .reshape(B, H, S, E).astype(np.float32, copy=False))


def kernel(**inputs):
    nc = _build()
    maps = _in_maps(inputs)
    res = bass_utils.run_bass_kernel_spmd(nc, maps, core_ids=list(range(NCORES)))
    return _gather(res.results)


if __name__ == "__main__":
    _build()
    print("build+compile OK")
